# revision 4
# baseline (speedup 1.0000x reference)
"""Corotational 2D beam (Euler-Bernoulli) message-passing kernel for 8x Trainium2 NeuronCores.

Strategy (edge sharding, per spec sharding_hint):
  - 4M elements sharded across 8 cores (500K each, padded to 128*3907).
  - Host marshals inputs: packs per-node [cx, cz, ux, uz, th] rows and expands
    them per edge endpoint (sharding/layout step), slices per-edge props.
  - Device (Bass/Tile kernel per core) computes ALL per-element arithmetic:
    geometry (l0, c, s), local frame rotation (d_local), Euler-Bernoulli end
    forces (f_local), global end forces (F_global_A/B), M_mid — streamed
    through SBUF in [128 x K] tiles. Work is split across DVE (tensor-tensor),
    ACT (sqrt/squares/scales/copies) and GpSimd (secondary tensor-tensor lane).
  - Host unshards: concatenates per-core outputs, extracts the duplicate
    column outputs (N_e=f3, V_e=f4, M1_e=f2, M2_e=f5 are defined as copies),
    and reduces the scattered nodal forces across shards (bincount = the
    all-reduce/psum step of the hint).
"""
import numpy as np

import concourse.bass as bass
import concourse.bacc as bacc
import concourse.mybir as mybir
import concourse.tile as tile
from concourse.bass_utils import run_bass_kernel_spmd

N_NODES = 1_000_000
N_ELEMS = 4_000_000
N_CORES = 8
P = 128
E_CORE = N_ELEMS // N_CORES          # 500_000
KTOT = -(-E_CORE // P)               # 3907 columns per partition
E_PAD = P * KTOT                     # 500_096
KTILE = 512

f32 = mybir.dt.float32
OP = mybir.AluOpType
AF = mybir.ActivationFunctionType

_PROFILE = False
LAST_EXEC_NS = None
LAST_MEAN_EXEC_NS = None

_CACHED_NC = None


def _build_module():
    nc = bacc.Bacc(None, target_bir_lowering=False)

    gA = nc.declare_dram_parameter("gA", [P, KTOT, 5], f32, isOutput=False)
    gB = nc.declare_dram_parameter("gB", [P, KTOT, 5], f32, isOutput=False)
    pE = nc.declare_dram_parameter("pE", [P, KTOT], f32, isOutput=False)
    pA = nc.declare_dram_parameter("pA", [P, KTOT], f32, isOutput=False)
    pI = nc.declare_dram_parameter("pI", [P, KTOT], f32, isOutput=False)

    # plane-major outputs: [P, width, KTOT]
    floc = nc.declare_dram_parameter("floc", [P, 6, KTOT], f32, isOutput=True)
    dloc = nc.declare_dram_parameter("dloc", [P, 6, KTOT], f32, isOutput=True)
    FAo = nc.declare_dram_parameter("FAo", [P, 3, KTOT], f32, isOutput=True)
    FBo = nc.declare_dram_parameter("FBo", [P, 3, KTOT], f32, isOutput=True)
    l0o = nc.declare_dram_parameter("l0o", [P, KTOT], f32, isOutput=True)
    co = nc.declare_dram_parameter("co", [P, KTOT], f32, isOutput=True)
    so = nc.declare_dram_parameter("so", [P, KTOT], f32, isOutput=True)
    mmo = nc.declare_dram_parameter("mmo", [P, KTOT], f32, isOutput=True)

    tiles = []
    k0 = 0
    while k0 < KTOT:
        kt = min(KTILE, KTOT - k0)
        tiles.append((k0, kt))
        k0 += kt

    with tile.TileContext(nc) as tc:
        with (
            tc.tile_pool(name="io", bufs=2) as io,
            tc.tile_pool(name="scr", bufs=1) as scr,
        ):
            for (k0, K) in tiles:
                sl = slice(k0, k0 + K)
                # ---- inputs ----
                gAt = io.tile([P, K, 5], f32, tag="gAt")
                gBt = io.tile([P, K, 5], f32, tag="gBt")
                pEt = io.tile([P, K], f32, tag="pEt")
                pAt = io.tile([P, K], f32, tag="pAt")
                pIt = io.tile([P, K], f32, tag="pIt")
                nc.sync.dma_start(out=gAt[:], in_=gA[:, sl, :])
                nc.sync.dma_start(out=gBt[:], in_=gB[:, sl, :])
                nc.sync.dma_start(out=pEt[:], in_=pE[:, sl])
                nc.sync.dma_start(out=pAt[:], in_=pA[:, sl])
                nc.sync.dma_start(out=pIt[:], in_=pI[:, sl])

                # ---- outputs (plane-major in SBUF) ----
                floct = io.tile([P, 6, K], f32, tag="floct")
                dlocA = io.tile([P, 3, K], f32, tag="dlocA")   # ua, wa, ta
                dlocB = io.tile([P, 3, K], f32, tag="dlocB")   # ub, wb, tb
                FAt = io.tile([P, 3, K], f32, tag="FAt")
                FBt = io.tile([P, 3, K], f32, tag="FBt")
                l0t = io.tile([P, K], f32, tag="l0t")
                ct = io.tile([P, K], f32, tag="ct")
                st = io.tile([P, K], f32, tag="st")
                mmt = io.tile([P, K], f32, tag="mmt")

                # ---- scratch ----
                names = ["dx", "dz", "t1", "t2", "sq", "inv", "i2", "e2",
                         "t12", "EAL", "EIL", "EX2", "wab", "tsum", "uu",
                         "vv", "hh", "du", "gt1", "gt2", "ms"]
                s_ = {n: scr.tile([P, K], f32, tag=n, name=n) for n in names}
                dx, dz, t1, t2, sq = s_["dx"], s_["dz"], s_["t1"], s_["t2"], s_["sq"]
                inv, i2, e2, t12 = s_["inv"], s_["i2"], s_["e2"], s_["t12"]
                EAL, EIL, EX2 = s_["EAL"], s_["EIL"], s_["EX2"]
                wab, tsum, uu, vv = s_["wab"], s_["tsum"], s_["uu"], s_["vv"]
                hh, du, gt1, gt2, ms = s_["hh"], s_["du"], s_["gt1"], s_["gt2"], s_["ms"]

                cxA, czA = gAt[:, :, 0], gAt[:, :, 1]
                uxA, uzA, thA = gAt[:, :, 2], gAt[:, :, 3], gAt[:, :, 4]
                cxB, czB = gBt[:, :, 0], gBt[:, :, 1]
                uxB, uzB, thB = gBt[:, :, 2], gBt[:, :, 3], gBt[:, :, 4]

                V = nc.vector
                S = nc.scalar
                G = nc.gpsimd

                # geometry: dx/dz on GpSimd, squares on ACT, combine on DVE
                G.tensor_tensor(out=dx[:], in0=cxB, in1=cxA, op=OP.subtract)
                G.tensor_tensor(out=dz[:], in0=czB, in1=czA, op=OP.subtract)
                S.activation(out=t1[:], in_=dx[:], func=AF.Square)
                S.activation(out=t2[:], in_=dz[:], func=AF.Square)
                V.tensor_tensor(out=sq[:], in0=t1[:], in1=t2[:], op=OP.add)
                S.sqrt(out=l0t[:], in_=sq[:])
                V.reciprocal_approx_accurate(out=inv[:], in_=l0t[:], scratch=t1[:])
                V.tensor_tensor(out=ct[:], in0=dx[:], in1=inv[:], op=OP.mult)
                V.tensor_tensor(out=st[:], in0=dz[:], in1=inv[:], op=OP.mult)
                S.activation(out=i2[:], in_=inv[:], func=AF.Square)

                # stiffness scalars
                V.tensor_tensor(out=t1[:], in0=pEt[:], in1=pAt[:], op=OP.mult)  # EA
                V.tensor_tensor(out=EAL[:], in0=t1[:], in1=inv[:], op=OP.mult)  # EA/L
                V.tensor_tensor(out=t2[:], in0=pEt[:], in1=pIt[:], op=OP.mult)  # EI
                V.tensor_tensor(out=EIL[:], in0=t2[:], in1=inv[:], op=OP.mult)  # EI/L
                V.tensor_tensor(out=e2[:], in0=t2[:], in1=i2[:], op=OP.mult)    # EI/L2
                V.scalar_tensor_tensor(out=t12[:], in0=e2[:], scalar=12.0, in1=inv[:],
                                       op0=OP.mult, op1=OP.mult)                # 12 EI/L3
                S.mul(out=e2[:], in_=e2[:], mul=6.0)                            # 6 EI/L2
                S.mul(out=EX2[:], in_=EIL[:], mul=2.0)                          # 2 EI/L

                # local frame: A side on DVE, B side on GpSimd
                V.tensor_tensor(out=t1[:], in0=ct[:], in1=uxA, op=OP.mult)
                V.tensor_tensor(out=t2[:], in0=st[:], in1=uzA, op=OP.mult)
                V.tensor_tensor(out=dlocA[:, 0, :], in0=t1[:], in1=t2[:], op=OP.add)       # ua
                V.tensor_tensor(out=t1[:], in0=ct[:], in1=uzA, op=OP.mult)
                V.tensor_tensor(out=t2[:], in0=st[:], in1=uxA, op=OP.mult)
                V.tensor_tensor(out=dlocA[:, 1, :], in0=t1[:], in1=t2[:], op=OP.subtract)  # wa
                S.copy(out=dlocA[:, 2, :], in_=thA)                                        # ta
                G.tensor_tensor(out=gt1[:], in0=ct[:], in1=uxB, op=OP.mult)
                G.tensor_tensor(out=gt2[:], in0=st[:], in1=uzB, op=OP.mult)
                G.tensor_tensor(out=dlocB[:, 0, :], in0=gt1[:], in1=gt2[:], op=OP.add)     # ub
                G.tensor_tensor(out=gt1[:], in0=ct[:], in1=uzB, op=OP.mult)
                G.tensor_tensor(out=gt2[:], in0=st[:], in1=uxB, op=OP.mult)
                G.tensor_tensor(out=dlocB[:, 1, :], in0=gt1[:], in1=gt2[:], op=OP.subtract)  # wb
                S.copy(out=dlocB[:, 2, :], in_=thB)                                        # tb

                # theta combinations on GpSimd
                G.tensor_tensor(out=tsum[:], in0=thA, in1=thB, op=OP.add)
                G.tensor_tensor(out=uu[:], in0=tsum[:], in1=thA, op=OP.add)  # 2ta+tb
                G.tensor_tensor(out=vv[:], in0=tsum[:], in1=thB, op=OP.add)  # ta+2tb

                # f0 = EA/L * (ua - ub); f3 = -f0
                V.tensor_tensor(out=du[:], in0=dlocA[:, 0, :], in1=dlocB[:, 0, :], op=OP.subtract)
                V.tensor_tensor(out=floct[:, 0, :], in0=EAL[:], in1=du[:], op=OP.mult)
                S.mul(out=floct[:, 3, :], in_=floct[:, 0, :], mul=-1.0)

                # f1 = 12EI/L3*(wa-wb) + 6EI/L2*(ta+tb); f4 = -f1
                V.tensor_tensor(out=wab[:], in0=dlocA[:, 1, :], in1=dlocB[:, 1, :], op=OP.subtract)
                V.tensor_tensor(out=t1[:], in0=t12[:], in1=wab[:], op=OP.mult)
                V.tensor_tensor(out=t2[:], in0=e2[:], in1=tsum[:], op=OP.mult)
                V.tensor_tensor(out=floct[:, 1, :], in0=t1[:], in1=t2[:], op=OP.add)
                S.mul(out=floct[:, 4, :], in_=floct[:, 1, :], mul=-1.0)

                # f2 = h + (2EI/L)*(2ta+tb);  f5 = h + (2EI/L)*(ta+2tb);  h = 6EI/L2*(wa-wb)
                V.tensor_tensor(out=hh[:], in0=e2[:], in1=wab[:], op=OP.mult)
                V.tensor_tensor(out=t1[:], in0=EX2[:], in1=uu[:], op=OP.mult)
                V.tensor_tensor(out=floct[:, 2, :], in0=hh[:], in1=t1[:], op=OP.add)
                V.tensor_tensor(out=t2[:], in0=EX2[:], in1=vv[:], op=OP.mult)
                V.tensor_tensor(out=floct[:, 5, :], in0=hh[:], in1=t2[:], op=OP.add)

                # M_mid = (f5 - f2) * 0.5   (sub on GpSimd, scale on ACT)
                G.tensor_tensor(out=ms[:], in0=floct[:, 5, :], in1=floct[:, 2, :], op=OP.subtract)
                S.mul(out=mmt[:], in_=ms[:], mul=0.5)

                # F_global_A = [c*f0 - s*f1, s*f0 + c*f1, f2]
                V.tensor_tensor(out=t1[:], in0=ct[:], in1=floct[:, 0, :], op=OP.mult)
                V.tensor_tensor(out=t2[:], in0=st[:], in1=floct[:, 1, :], op=OP.mult)
                V.tensor_tensor(out=FAt[:, 0, :], in0=t1[:], in1=t2[:], op=OP.subtract)
                V.tensor_tensor(out=t1[:], in0=st[:], in1=floct[:, 0, :], op=OP.mult)
                V.tensor_tensor(out=t2[:], in0=ct[:], in1=floct[:, 1, :], op=OP.mult)
                V.tensor_tensor(out=FAt[:, 1, :], in0=t1[:], in1=t2[:], op=OP.add)
                S.copy(out=FAt[:, 2, :], in_=floct[:, 2, :])
                # F_global_B = [-FA0, -FA1, f5] (exact: f3=-f0, f4=-f1)
                S.mul(out=FBt[:, 0, :], in_=FAt[:, 0, :], mul=-1.0)
                S.mul(out=FBt[:, 1, :], in_=FAt[:, 1, :], mul=-1.0)
                S.copy(out=FBt[:, 2, :], in_=floct[:, 5, :])

                # ---- store ----
                nc.scalar.dma_start(out=floc[:, :, sl], in_=floct[:])
                nc.scalar.dma_start(out=dloc[:, 0:3, sl], in_=dlocA[:])
                nc.scalar.dma_start(out=dloc[:, 3:6, sl], in_=dlocB[:])
                nc.scalar.dma_start(out=FAo[:, :, sl], in_=FAt[:])
                nc.scalar.dma_start(out=FBo[:, :, sl], in_=FBt[:])
                nc.scalar.dma_start(out=l0o[:, sl], in_=l0t[:])
                nc.scalar.dma_start(out=co[:, sl], in_=ct[:])
                nc.scalar.dma_start(out=so[:, sl], in_=st[:])
                nc.scalar.dma_start(out=mmo[:, sl], in_=mmt[:])

    nc.compile()
    return nc


def _get_nc():
    global _CACHED_NC
    if _CACHED_NC is None:
        _CACHED_NC = _build_module()
    return _CACHED_NC


def kernel(pred_disp, connectivity, coords, prop_E, prop_A, prop_I22):
    global LAST_EXEC_NS, LAST_MEAN_EXEC_NS
    pred_disp = np.asarray(pred_disp, dtype=np.float32)
    coords = np.asarray(coords, dtype=np.float32)
    conn = np.asarray(connectivity)
    prop_E = np.asarray(prop_E, dtype=np.float32)
    prop_A = np.asarray(prop_A, dtype=np.float32)
    prop_I22 = np.asarray(prop_I22, dtype=np.float32)

    nA = conn[:, 0]
    nB = conn[:, 1]

    # ---- shard + marshal inputs (host): packed node rows expanded per edge ----
    node_pack = np.empty((N_NODES, 5), np.float32)
    node_pack[:, 0] = coords[:, 0]
    node_pack[:, 1] = coords[:, 2]
    node_pack[:, 2:5] = pred_disp

    gApad = np.zeros((N_CORES, E_PAD, 5), np.float32)
    gBpad = np.zeros((N_CORES, E_PAD, 5), np.float32)
    pEpad = np.ones((N_CORES, E_PAD), np.float32)
    pApad = np.ones((N_CORES, E_PAD), np.float32)
    pIpad = np.ones((N_CORES, E_PAD), np.float32)
    for c in range(N_CORES):
        cs, ce = c * E_CORE, (c + 1) * E_CORE
        np.take(node_pack, nA[cs:ce], axis=0, out=gApad[c, :E_CORE], mode="clip")
        np.take(node_pack, nB[cs:ce], axis=0, out=gBpad[c, :E_CORE], mode="clip")
        pEpad[c, :E_CORE] = prop_E[cs:ce]
        pApad[c, :E_CORE] = prop_A[cs:ce]
        pIpad[c, :E_CORE] = prop_I22[cs:ce]
    gBpad[:, E_CORE:, 0] = 1.0  # padding rows: dx=1, dz=0 -> finite everywhere

    in_maps = [
        {
            "gA": gApad[c].reshape(P, KTOT, 5),
            "gB": gBpad[c].reshape(P, KTOT, 5),
            "pE": pEpad[c].reshape(P, KTOT),
            "pA": pApad[c].reshape(P, KTOT),
            "pI": pIpad[c].reshape(P, KTOT),
        }
        for c in range(N_CORES)
    ]

    nc = _get_nc()
    res = run_bass_kernel_spmd(nc, in_maps, core_ids=list(range(N_CORES)),
                               trace=_PROFILE)
    LAST_EXEC_NS = res.exec_time_ns
    LAST_MEAN_EXEC_NS = res.mean_exec_time_ns

    # ---- unshard outputs (plane-major device layout -> row-major arrays) ----
    f_local = np.empty((N_ELEMS, 6), np.float32)
    d_local = np.empty((N_ELEMS, 6), np.float32)
    F_A = np.empty((N_ELEMS, 3), np.float32)
    F_B = np.empty((N_ELEMS, 3), np.float32)
    l0 = np.empty(N_ELEMS, np.float32)
    c_ = np.empty(N_ELEMS, np.float32)
    s_ = np.empty(N_ELEMS, np.float32)
    M_mid = np.empty(N_ELEMS, np.float32)
    for c in range(N_CORES):
        cs, ce = c * E_CORE, (c + 1) * E_CORE
        r = res.results[c]
        f_local[cs:ce] = r["floc"].transpose(0, 2, 1).reshape(E_PAD, 6)[:E_CORE]
        d_local[cs:ce] = r["dloc"].transpose(0, 2, 1).reshape(E_PAD, 6)[:E_CORE]
        F_A[cs:ce] = r["FAo"].transpose(0, 2, 1).reshape(E_PAD, 3)[:E_CORE]
        F_B[cs:ce] = r["FBo"].transpose(0, 2, 1).reshape(E_PAD, 3)[:E_CORE]
        l0[cs:ce] = r["l0o"].reshape(E_PAD)[:E_CORE]
        c_[cs:ce] = r["co"].reshape(E_PAD)[:E_CORE]
        s_[cs:ce] = r["so"].reshape(E_PAD)[:E_CORE]
        M_mid[cs:ce] = r["mmo"].reshape(E_PAD)[:E_CORE]

    # duplicate-column outputs (defined as copies of f_local columns)
    N_e = np.ascontiguousarray(f_local[:, 3])
    M1_e = np.ascontiguousarray(f_local[:, 2])
    M2_e = np.ascontiguousarray(f_local[:, 5])
    V_e = np.ascontiguousarray(f_local[:, 4])

    # all-reduce of the scattered nodal forces across edge shards
    nodal_forces = np.zeros((N_NODES, 3), np.float32)
    for comp in range(3):
        acc = np.bincount(nA, weights=F_A[:, comp], minlength=N_NODES)
        acc += np.bincount(nB, weights=F_B[:, comp], minlength=N_NODES)
        nodal_forces[:, comp] = acc

    return (nodal_forces, f_local, d_local, F_A, F_B,
            N_e, M_mid, V_e, M1_e, M2_e, l0, c_, s_)


# revision 6
# speedup vs baseline: 1.2088x; 1.2088x over previous
"""Corotational 2D beam (Euler-Bernoulli) message-passing kernel for 8x Trainium2 NeuronCores.

Strategy (edge sharding, per spec sharding_hint):
  - 4M elements sharded across 8 cores (500K each, padded to 128*3907).
  - Host marshals inputs: packs per-node [cx, cz, ux, uz, th] rows and expands
    them per edge endpoint (sharding/layout step), slices per-edge props.
  - Device (Bass/Tile kernel per core) computes ALL per-element arithmetic:
    geometry (l0, c, s), local frame rotation (d_local), Euler-Bernoulli end
    forces (f_local), global end forces (F_global_A/B), M_mid — streamed
    through SBUF in [128 x K] tiles. Work is split across DVE (tensor-tensor),
    ACT (sqrt/squares/scales/copies) and GpSimd (secondary tensor-tensor lane).
  - Host unshards: concatenates per-core outputs, extracts the duplicate
    column outputs (N_e=f3, V_e=f4, M1_e=f2, M2_e=f5 are defined as copies),
    and reduces the scattered nodal forces across shards (bincount = the
    all-reduce/psum step of the hint).
"""
import numpy as np

import concourse.bass as bass
import concourse.bacc as bacc
import concourse.mybir as mybir
import concourse.tile as tile
from concourse.bass_utils import run_bass_kernel_spmd

N_NODES = 1_000_000
N_ELEMS = 4_000_000
N_CORES = 8
P = 128
E_CORE = N_ELEMS // N_CORES          # 500_000
KTOT = -(-E_CORE // P)               # 3907 columns per partition
E_PAD = P * KTOT                     # 500_096
KTILE = 512

f32 = mybir.dt.float32
OP = mybir.AluOpType
AF = mybir.ActivationFunctionType

_PROFILE = False
LAST_EXEC_NS = None
LAST_MEAN_EXEC_NS = None

_CACHED_NC = None


def _build_module():
    nc = bacc.Bacc(None, target_bir_lowering=False)

    gA = nc.declare_dram_parameter("gA", [P, KTOT, 5], f32, isOutput=False)
    gB = nc.declare_dram_parameter("gB", [P, KTOT, 5], f32, isOutput=False)
    pE = nc.declare_dram_parameter("pE", [P, KTOT], f32, isOutput=False)
    pA = nc.declare_dram_parameter("pA", [P, KTOT], f32, isOutput=False)
    pI = nc.declare_dram_parameter("pI", [P, KTOT], f32, isOutput=False)

    # plane-major outputs: [P, width, KTOT]
    floc = nc.declare_dram_parameter("floc", [P, 6, KTOT], f32, isOutput=True)
    dloc = nc.declare_dram_parameter("dloc", [P, 6, KTOT], f32, isOutput=True)
    FAo = nc.declare_dram_parameter("FAo", [P, 3, KTOT], f32, isOutput=True)
    FBo = nc.declare_dram_parameter("FBo", [P, 3, KTOT], f32, isOutput=True)
    l0o = nc.declare_dram_parameter("l0o", [P, KTOT], f32, isOutput=True)
    co = nc.declare_dram_parameter("co", [P, KTOT], f32, isOutput=True)
    so = nc.declare_dram_parameter("so", [P, KTOT], f32, isOutput=True)
    mmo = nc.declare_dram_parameter("mmo", [P, KTOT], f32, isOutput=True)

    tiles = []
    k0 = 0
    while k0 < KTOT:
        kt = min(KTILE, KTOT - k0)
        tiles.append((k0, kt))
        k0 += kt

    with tile.TileContext(nc) as tc:
        with (
            tc.tile_pool(name="io", bufs=2) as io,
            tc.tile_pool(name="scr", bufs=1) as scr,
        ):
            for (k0, K) in tiles:
                sl = slice(k0, k0 + K)
                # ---- inputs ----
                gAt = io.tile([P, K, 5], f32, tag="gAt")
                gBt = io.tile([P, K, 5], f32, tag="gBt")
                pEt = io.tile([P, K], f32, tag="pEt")
                pAt = io.tile([P, K], f32, tag="pAt")
                pIt = io.tile([P, K], f32, tag="pIt")
                nc.sync.dma_start(out=gAt[:], in_=gA[:, sl, :])
                nc.sync.dma_start(out=gBt[:], in_=gB[:, sl, :])
                nc.sync.dma_start(out=pEt[:], in_=pE[:, sl])
                nc.sync.dma_start(out=pAt[:], in_=pA[:, sl])
                nc.sync.dma_start(out=pIt[:], in_=pI[:, sl])

                # ---- outputs (plane-major in SBUF) ----
                floct = io.tile([P, 6, K], f32, tag="floct")
                dlocA = io.tile([P, 3, K], f32, tag="dlocA")   # ua, wa, ta
                dlocB = io.tile([P, 3, K], f32, tag="dlocB")   # ub, wb, tb
                FAt = io.tile([P, 3, K], f32, tag="FAt")
                FBt = io.tile([P, 3, K], f32, tag="FBt")
                l0t = io.tile([P, K], f32, tag="l0t")
                ct = io.tile([P, K], f32, tag="ct")
                st = io.tile([P, K], f32, tag="st")
                mmt = io.tile([P, K], f32, tag="mmt")

                # ---- scratch ----
                names = ["dx", "dz", "t1", "t2", "sq", "inv", "i2", "e2",
                         "t12", "EAL", "EIL", "EX2", "wab", "tsum", "uu",
                         "vv", "hh", "du", "gt1", "gt2", "ms"]
                s_ = {n: scr.tile([P, K], f32, tag=n, name=n) for n in names}
                dx, dz, t1, t2, sq = s_["dx"], s_["dz"], s_["t1"], s_["t2"], s_["sq"]
                inv, i2, e2, t12 = s_["inv"], s_["i2"], s_["e2"], s_["t12"]
                EAL, EIL, EX2 = s_["EAL"], s_["EIL"], s_["EX2"]
                wab, tsum, uu, vv = s_["wab"], s_["tsum"], s_["uu"], s_["vv"]
                hh, du, gt1, gt2, ms = s_["hh"], s_["du"], s_["gt1"], s_["gt2"], s_["ms"]

                cxA, czA = gAt[:, :, 0], gAt[:, :, 1]
                uxA, uzA, thA = gAt[:, :, 2], gAt[:, :, 3], gAt[:, :, 4]
                cxB, czB = gBt[:, :, 0], gBt[:, :, 1]
                uxB, uzB, thB = gBt[:, :, 2], gBt[:, :, 3], gBt[:, :, 4]

                V = nc.vector
                S = nc.scalar

                # geometry: squares on ACT, everything tensor-tensor on DVE
                V.tensor_tensor(out=dx[:], in0=cxB, in1=cxA, op=OP.subtract)
                V.tensor_tensor(out=dz[:], in0=czB, in1=czA, op=OP.subtract)
                S.activation(out=t1[:], in_=dx[:], func=AF.Square)
                S.activation(out=t2[:], in_=dz[:], func=AF.Square)
                V.tensor_tensor(out=sq[:], in0=t1[:], in1=t2[:], op=OP.add)
                S.sqrt(out=l0t[:], in_=sq[:])
                V.reciprocal_approx_accurate(out=inv[:], in_=l0t[:], scratch=t1[:])
                V.tensor_tensor(out=ct[:], in0=dx[:], in1=inv[:], op=OP.mult)
                V.tensor_tensor(out=st[:], in0=dz[:], in1=inv[:], op=OP.mult)
                S.activation(out=i2[:], in_=inv[:], func=AF.Square)

                # stiffness scalars
                V.tensor_tensor(out=t1[:], in0=pEt[:], in1=pAt[:], op=OP.mult)  # EA
                V.tensor_tensor(out=EAL[:], in0=t1[:], in1=inv[:], op=OP.mult)  # EA/L
                V.tensor_tensor(out=t2[:], in0=pEt[:], in1=pIt[:], op=OP.mult)  # EI
                V.tensor_tensor(out=EIL[:], in0=t2[:], in1=inv[:], op=OP.mult)  # EI/L
                V.tensor_tensor(out=e2[:], in0=t2[:], in1=i2[:], op=OP.mult)    # EI/L2
                V.scalar_tensor_tensor(out=t12[:], in0=e2[:], scalar=12.0, in1=inv[:],
                                       op0=OP.mult, op1=OP.mult)                # 12 EI/L3
                S.mul(out=e2[:], in_=e2[:], mul=6.0)                            # 6 EI/L2
                S.mul(out=EX2[:], in_=EIL[:], mul=2.0)                          # 2 EI/L

                # local frame: A side on DVE, B side on GpSimd
                V.tensor_tensor(out=t1[:], in0=ct[:], in1=uxA, op=OP.mult)
                V.tensor_tensor(out=t2[:], in0=st[:], in1=uzA, op=OP.mult)
                V.tensor_tensor(out=dlocA[:, 0, :], in0=t1[:], in1=t2[:], op=OP.add)       # ua
                V.tensor_tensor(out=t1[:], in0=ct[:], in1=uzA, op=OP.mult)
                V.tensor_tensor(out=t2[:], in0=st[:], in1=uxA, op=OP.mult)
                V.tensor_tensor(out=dlocA[:, 1, :], in0=t1[:], in1=t2[:], op=OP.subtract)  # wa
                S.copy(out=dlocA[:, 2, :], in_=thA)                                        # ta
                V.tensor_tensor(out=gt1[:], in0=ct[:], in1=uxB, op=OP.mult)
                V.tensor_tensor(out=gt2[:], in0=st[:], in1=uzB, op=OP.mult)
                V.tensor_tensor(out=dlocB[:, 0, :], in0=gt1[:], in1=gt2[:], op=OP.add)     # ub
                V.tensor_tensor(out=gt1[:], in0=ct[:], in1=uzB, op=OP.mult)
                V.tensor_tensor(out=gt2[:], in0=st[:], in1=uxB, op=OP.mult)
                V.tensor_tensor(out=dlocB[:, 1, :], in0=gt1[:], in1=gt2[:], op=OP.subtract)  # wb
                S.copy(out=dlocB[:, 2, :], in_=thB)                                        # tb

                # theta combinations
                V.tensor_tensor(out=tsum[:], in0=thA, in1=thB, op=OP.add)
                V.scalar_tensor_tensor(out=uu[:], in0=thA, scalar=2.0, in1=thB,
                                       op0=OP.mult, op1=OP.add)   # 2ta+tb
                V.scalar_tensor_tensor(out=vv[:], in0=thB, scalar=2.0, in1=thA,
                                       op0=OP.mult, op1=OP.add)   # ta+2tb

                # f0 = EA/L * (ua - ub); f3 = -f0
                V.tensor_tensor(out=du[:], in0=dlocA[:, 0, :], in1=dlocB[:, 0, :], op=OP.subtract)
                V.tensor_tensor(out=floct[:, 0, :], in0=EAL[:], in1=du[:], op=OP.mult)
                S.mul(out=floct[:, 3, :], in_=floct[:, 0, :], mul=-1.0)

                # f1 = 12EI/L3*(wa-wb) + 6EI/L2*(ta+tb); f4 = -f1
                V.tensor_tensor(out=wab[:], in0=dlocA[:, 1, :], in1=dlocB[:, 1, :], op=OP.subtract)
                V.tensor_tensor(out=t1[:], in0=t12[:], in1=wab[:], op=OP.mult)
                V.tensor_tensor(out=t2[:], in0=e2[:], in1=tsum[:], op=OP.mult)
                V.tensor_tensor(out=floct[:, 1, :], in0=t1[:], in1=t2[:], op=OP.add)
                S.mul(out=floct[:, 4, :], in_=floct[:, 1, :], mul=-1.0)

                # f2 = h + (2EI/L)*(2ta+tb);  f5 = h + (2EI/L)*(ta+2tb);  h = 6EI/L2*(wa-wb)
                V.tensor_tensor(out=hh[:], in0=e2[:], in1=wab[:], op=OP.mult)
                V.tensor_tensor(out=t1[:], in0=EX2[:], in1=uu[:], op=OP.mult)
                V.tensor_tensor(out=floct[:, 2, :], in0=hh[:], in1=t1[:], op=OP.add)
                V.tensor_tensor(out=t2[:], in0=EX2[:], in1=vv[:], op=OP.mult)
                V.tensor_tensor(out=floct[:, 5, :], in0=hh[:], in1=t2[:], op=OP.add)

                # M_mid = (f5 - f2) * 0.5
                V.tensor_tensor(out=ms[:], in0=floct[:, 5, :], in1=floct[:, 2, :], op=OP.subtract)
                S.mul(out=mmt[:], in_=ms[:], mul=0.5)

                # F_global_A = [c*f0 - s*f1, s*f0 + c*f1, f2]
                V.tensor_tensor(out=t1[:], in0=ct[:], in1=floct[:, 0, :], op=OP.mult)
                V.tensor_tensor(out=t2[:], in0=st[:], in1=floct[:, 1, :], op=OP.mult)
                V.tensor_tensor(out=FAt[:, 0, :], in0=t1[:], in1=t2[:], op=OP.subtract)
                V.tensor_tensor(out=t1[:], in0=st[:], in1=floct[:, 0, :], op=OP.mult)
                V.tensor_tensor(out=t2[:], in0=ct[:], in1=floct[:, 1, :], op=OP.mult)
                V.tensor_tensor(out=FAt[:, 1, :], in0=t1[:], in1=t2[:], op=OP.add)
                S.copy(out=FAt[:, 2, :], in_=floct[:, 2, :])
                # F_global_B = [-FA0, -FA1, f5] (exact: f3=-f0, f4=-f1)
                S.mul(out=FBt[:, 0, :], in_=FAt[:, 0, :], mul=-1.0)
                S.mul(out=FBt[:, 1, :], in_=FAt[:, 1, :], mul=-1.0)
                S.copy(out=FBt[:, 2, :], in_=floct[:, 5, :])

                # ---- store ----
                nc.scalar.dma_start(out=floc[:, :, sl], in_=floct[:])
                nc.scalar.dma_start(out=dloc[:, 0:3, sl], in_=dlocA[:])
                nc.scalar.dma_start(out=dloc[:, 3:6, sl], in_=dlocB[:])
                nc.scalar.dma_start(out=FAo[:, :, sl], in_=FAt[:])
                nc.scalar.dma_start(out=FBo[:, :, sl], in_=FBt[:])
                nc.scalar.dma_start(out=l0o[:, sl], in_=l0t[:])
                nc.scalar.dma_start(out=co[:, sl], in_=ct[:])
                nc.scalar.dma_start(out=so[:, sl], in_=st[:])
                nc.scalar.dma_start(out=mmo[:, sl], in_=mmt[:])

    nc.compile()
    return nc


def _get_nc():
    global _CACHED_NC
    if _CACHED_NC is None:
        _CACHED_NC = _build_module()
    return _CACHED_NC


def kernel(pred_disp, connectivity, coords, prop_E, prop_A, prop_I22):
    global LAST_EXEC_NS, LAST_MEAN_EXEC_NS
    pred_disp = np.asarray(pred_disp, dtype=np.float32)
    coords = np.asarray(coords, dtype=np.float32)
    conn = np.asarray(connectivity)
    prop_E = np.asarray(prop_E, dtype=np.float32)
    prop_A = np.asarray(prop_A, dtype=np.float32)
    prop_I22 = np.asarray(prop_I22, dtype=np.float32)

    nA = conn[:, 0]
    nB = conn[:, 1]

    # ---- shard + marshal inputs (host): packed node rows expanded per edge ----
    node_pack = np.empty((N_NODES, 5), np.float32)
    node_pack[:, 0] = coords[:, 0]
    node_pack[:, 1] = coords[:, 2]
    node_pack[:, 2:5] = pred_disp

    gApad = np.zeros((N_CORES, E_PAD, 5), np.float32)
    gBpad = np.zeros((N_CORES, E_PAD, 5), np.float32)
    pEpad = np.ones((N_CORES, E_PAD), np.float32)
    pApad = np.ones((N_CORES, E_PAD), np.float32)
    pIpad = np.ones((N_CORES, E_PAD), np.float32)
    for c in range(N_CORES):
        cs, ce = c * E_CORE, (c + 1) * E_CORE
        np.take(node_pack, nA[cs:ce], axis=0, out=gApad[c, :E_CORE], mode="clip")
        np.take(node_pack, nB[cs:ce], axis=0, out=gBpad[c, :E_CORE], mode="clip")
        pEpad[c, :E_CORE] = prop_E[cs:ce]
        pApad[c, :E_CORE] = prop_A[cs:ce]
        pIpad[c, :E_CORE] = prop_I22[cs:ce]
    gBpad[:, E_CORE:, 0] = 1.0  # padding rows: dx=1, dz=0 -> finite everywhere

    in_maps = [
        {
            "gA": gApad[c].reshape(P, KTOT, 5),
            "gB": gBpad[c].reshape(P, KTOT, 5),
            "pE": pEpad[c].reshape(P, KTOT),
            "pA": pApad[c].reshape(P, KTOT),
            "pI": pIpad[c].reshape(P, KTOT),
        }
        for c in range(N_CORES)
    ]

    nc = _get_nc()
    res = run_bass_kernel_spmd(nc, in_maps, core_ids=list(range(N_CORES)),
                               trace=_PROFILE)
    LAST_EXEC_NS = res.exec_time_ns
    LAST_MEAN_EXEC_NS = res.mean_exec_time_ns

    # ---- unshard outputs (plane-major device layout -> row-major arrays) ----
    f_local = np.empty((N_ELEMS, 6), np.float32)
    d_local = np.empty((N_ELEMS, 6), np.float32)
    F_A = np.empty((N_ELEMS, 3), np.float32)
    F_B = np.empty((N_ELEMS, 3), np.float32)
    l0 = np.empty(N_ELEMS, np.float32)
    c_ = np.empty(N_ELEMS, np.float32)
    s_ = np.empty(N_ELEMS, np.float32)
    M_mid = np.empty(N_ELEMS, np.float32)
    for c in range(N_CORES):
        cs, ce = c * E_CORE, (c + 1) * E_CORE
        r = res.results[c]
        f_local[cs:ce] = r["floc"].transpose(0, 2, 1).reshape(E_PAD, 6)[:E_CORE]
        d_local[cs:ce] = r["dloc"].transpose(0, 2, 1).reshape(E_PAD, 6)[:E_CORE]
        F_A[cs:ce] = r["FAo"].transpose(0, 2, 1).reshape(E_PAD, 3)[:E_CORE]
        F_B[cs:ce] = r["FBo"].transpose(0, 2, 1).reshape(E_PAD, 3)[:E_CORE]
        l0[cs:ce] = r["l0o"].reshape(E_PAD)[:E_CORE]
        c_[cs:ce] = r["co"].reshape(E_PAD)[:E_CORE]
        s_[cs:ce] = r["so"].reshape(E_PAD)[:E_CORE]
        M_mid[cs:ce] = r["mmo"].reshape(E_PAD)[:E_CORE]

    # duplicate-column outputs (defined as copies of f_local columns)
    N_e = np.ascontiguousarray(f_local[:, 3])
    M1_e = np.ascontiguousarray(f_local[:, 2])
    M2_e = np.ascontiguousarray(f_local[:, 5])
    V_e = np.ascontiguousarray(f_local[:, 4])

    # all-reduce of the scattered nodal forces across edge shards
    nodal_forces = np.zeros((N_NODES, 3), np.float32)
    for comp in range(3):
        acc = np.bincount(nA, weights=F_A[:, comp], minlength=N_NODES)
        acc += np.bincount(nB, weights=F_B[:, comp], minlength=N_NODES)
        nodal_forces[:, comp] = acc

    return (nodal_forces, f_local, d_local, F_A, F_B,
            N_e, M_mid, V_e, M1_e, M2_e, l0, c_, s_)


# revision 10
# speedup vs baseline: 1.2441x; 1.0292x over previous
"""Corotational 2D beam (Euler-Bernoulli) message-passing kernel for 8x Trainium2 NeuronCores.

Strategy (edge sharding, per spec sharding_hint):
  - 4M elements sharded across 8 cores (500K each, padded to 128*3907).
  - Host marshals inputs: packs per-node [cx, cz, ux, uz, th] rows and expands
    them per edge endpoint (sharding/layout step), slices per-edge props.
  - Device (Bass/Tile kernel per core) computes ALL per-element arithmetic:
    geometry (l0, c, s), local frame rotation (d_local), Euler-Bernoulli end
    forces (f_local), global end forces (F_global_A/B), M_mid — streamed
    through SBUF in [128 x K] tiles. Work is split across DVE (tensor-tensor),
    ACT (sqrt/squares/scales/copies) and GpSimd (secondary tensor-tensor lane).
  - Host unshards: concatenates per-core outputs, extracts the duplicate
    column outputs (N_e=f3, V_e=f4, M1_e=f2, M2_e=f5 are defined as copies),
    and reduces the scattered nodal forces across shards (bincount = the
    all-reduce/psum step of the hint).
"""
import numpy as np

import concourse.bass as bass
import concourse.bacc as bacc
import concourse.mybir as mybir
import concourse.tile as tile
from concourse.bass_utils import run_bass_kernel_spmd

N_NODES = 1_000_000
N_ELEMS = 4_000_000
N_CORES = 8
P = 128
E_CORE = N_ELEMS // N_CORES          # 500_000
KTOT = -(-E_CORE // P)               # 3907 columns per partition
E_PAD = P * KTOT                     # 500_096
KTILE = 512

f32 = mybir.dt.float32
OP = mybir.AluOpType
AF = mybir.ActivationFunctionType

_PROFILE = False
LAST_EXEC_NS = None
LAST_MEAN_EXEC_NS = None

_CACHED_NC = None


def _build_module():
    nc = bacc.Bacc(None, target_bir_lowering=False)

    gA = nc.declare_dram_parameter("gA", [P, KTOT, 5], f32, isOutput=False)
    gB = nc.declare_dram_parameter("gB", [P, KTOT, 5], f32, isOutput=False)
    pE = nc.declare_dram_parameter("pE", [P, KTOT], f32, isOutput=False)
    pA = nc.declare_dram_parameter("pA", [P, KTOT], f32, isOutput=False)
    pI = nc.declare_dram_parameter("pI", [P, KTOT], f32, isOutput=False)

    # plane-major outputs: [P, width, KTOT]
    floc = nc.declare_dram_parameter("floc", [P, 6, KTOT], f32, isOutput=True)
    dloc = nc.declare_dram_parameter("dloc", [P, 6, KTOT], f32, isOutput=True)
    FAo = nc.declare_dram_parameter("FAo", [P, 3, KTOT], f32, isOutput=True)
    FBo = nc.declare_dram_parameter("FBo", [P, 3, KTOT], f32, isOutput=True)
    l0o = nc.declare_dram_parameter("l0o", [P, KTOT], f32, isOutput=True)
    co = nc.declare_dram_parameter("co", [P, KTOT], f32, isOutput=True)
    so = nc.declare_dram_parameter("so", [P, KTOT], f32, isOutput=True)
    mmo = nc.declare_dram_parameter("mmo", [P, KTOT], f32, isOutput=True)

    tiles = []
    k0 = 0
    while k0 < KTOT:
        kt = min(KTILE, KTOT - k0)
        tiles.append((k0, kt))
        k0 += kt

    with tile.TileContext(nc) as tc:
        with (
            tc.tile_pool(name="io", bufs=2) as io,
            tc.tile_pool(name="scr", bufs=1) as scr,
        ):
            for (k0, K) in tiles:
                sl = slice(k0, k0 + K)
                # ---- inputs ----
                gAt = io.tile([P, K, 5], f32, tag="gAt")
                gBt = io.tile([P, K, 5], f32, tag="gBt")
                pEt = io.tile([P, K], f32, tag="pEt")
                pAt = io.tile([P, K], f32, tag="pAt")
                pIt = io.tile([P, K], f32, tag="pIt")
                nc.sync.dma_start(out=gAt[:], in_=gA[:, sl, :])
                nc.sync.dma_start(out=gBt[:], in_=gB[:, sl, :])
                nc.sync.dma_start(out=pEt[:], in_=pE[:, sl])
                nc.sync.dma_start(out=pAt[:], in_=pA[:, sl])
                nc.sync.dma_start(out=pIt[:], in_=pI[:, sl])

                # ---- outputs (plane-major in SBUF) ----
                floct = io.tile([P, 6, K], f32, tag="floct")
                dlocA = io.tile([P, 3, K], f32, tag="dlocA")   # ua, wa, ta
                dlocB = io.tile([P, 3, K], f32, tag="dlocB")   # ub, wb, tb
                FAt = io.tile([P, 3, K], f32, tag="FAt")
                FBt = io.tile([P, 3, K], f32, tag="FBt")
                l0t = io.tile([P, K], f32, tag="l0t")
                ct = io.tile([P, K], f32, tag="ct")
                st = io.tile([P, K], f32, tag="st")
                mmt = io.tile([P, K], f32, tag="mmt")

                # ---- scratch ----
                names = ["dx", "dz", "t1", "t2", "sq", "inv", "i2", "e2",
                         "EAL", "EIL", "EX2", "wab", "tsum", "uu",
                         "vv", "hh", "du", "gt1", "gt2"]
                s_ = {n: scr.tile([P, K], f32, tag=n, name=n) for n in names}
                dx, dz, t1, t2, sq = s_["dx"], s_["dz"], s_["t1"], s_["t2"], s_["sq"]
                inv, i2, e2 = s_["inv"], s_["i2"], s_["e2"]
                EAL, EIL, EX2 = s_["EAL"], s_["EIL"], s_["EX2"]
                wab, tsum, uu, vv = s_["wab"], s_["tsum"], s_["uu"], s_["vv"]
                hh, du, gt1, gt2 = s_["hh"], s_["du"], s_["gt1"], s_["gt2"]

                cxA, czA = gAt[:, :, 0], gAt[:, :, 1]
                uxA, uzA, thA = gAt[:, :, 2], gAt[:, :, 3], gAt[:, :, 4]
                cxB, czB = gBt[:, :, 0], gBt[:, :, 1]
                uxB, uzB, thB = gBt[:, :, 2], gBt[:, :, 3], gBt[:, :, 4]

                V = nc.vector
                S = nc.scalar

                # geometry: squares on ACT, everything tensor-tensor on DVE
                V.tensor_tensor(out=dx[:], in0=cxB, in1=cxA, op=OP.subtract)
                V.tensor_tensor(out=dz[:], in0=czB, in1=czA, op=OP.subtract)
                S.activation(out=t1[:], in_=dx[:], func=AF.Square)
                S.activation(out=t2[:], in_=dz[:], func=AF.Square)
                V.tensor_tensor(out=sq[:], in0=t1[:], in1=t2[:], op=OP.add)
                S.sqrt(out=l0t[:], in_=sq[:])
                V.reciprocal_approx_accurate(out=inv[:], in_=l0t[:], scratch=t1[:])
                V.tensor_tensor(out=ct[:], in0=dx[:], in1=inv[:], op=OP.mult)
                V.tensor_tensor(out=st[:], in0=dz[:], in1=inv[:], op=OP.mult)
                S.activation(out=i2[:], in_=inv[:], func=AF.Square)

                # stiffness scalars
                V.tensor_tensor(out=t1[:], in0=pEt[:], in1=pAt[:], op=OP.mult)  # EA
                V.tensor_tensor(out=EAL[:], in0=t1[:], in1=inv[:], op=OP.mult)  # EA/L
                V.tensor_tensor(out=t2[:], in0=pEt[:], in1=pIt[:], op=OP.mult)  # EI
                V.tensor_tensor(out=EIL[:], in0=t2[:], in1=inv[:], op=OP.mult)  # EI/L
                V.tensor_tensor(out=e2[:], in0=t2[:], in1=i2[:], op=OP.mult)    # EI/L2
                S.mul(out=e2[:], in_=e2[:], mul=6.0)                            # 6 EI/L2
                S.mul(out=EX2[:], in_=EIL[:], mul=2.0)                          # 2 EI/L

                # local frame: A side on DVE, B side on GpSimd
                V.tensor_tensor(out=t1[:], in0=ct[:], in1=uxA, op=OP.mult)
                V.tensor_tensor(out=t2[:], in0=st[:], in1=uzA, op=OP.mult)
                V.tensor_tensor(out=dlocA[:, 0, :], in0=t1[:], in1=t2[:], op=OP.add)       # ua
                V.tensor_tensor(out=t1[:], in0=ct[:], in1=uzA, op=OP.mult)
                V.tensor_tensor(out=t2[:], in0=st[:], in1=uxA, op=OP.mult)
                V.tensor_tensor(out=dlocA[:, 1, :], in0=t1[:], in1=t2[:], op=OP.subtract)  # wa
                S.copy(out=dlocA[:, 2, :], in_=thA)                                        # ta
                V.tensor_tensor(out=gt1[:], in0=ct[:], in1=uxB, op=OP.mult)
                V.tensor_tensor(out=gt2[:], in0=st[:], in1=uzB, op=OP.mult)
                V.tensor_tensor(out=dlocB[:, 0, :], in0=gt1[:], in1=gt2[:], op=OP.add)     # ub
                V.tensor_tensor(out=gt1[:], in0=ct[:], in1=uzB, op=OP.mult)
                V.tensor_tensor(out=gt2[:], in0=st[:], in1=uxB, op=OP.mult)
                V.tensor_tensor(out=dlocB[:, 1, :], in0=gt1[:], in1=gt2[:], op=OP.subtract)  # wb
                S.copy(out=dlocB[:, 2, :], in_=thB)                                        # tb

                # theta combinations
                V.tensor_tensor(out=tsum[:], in0=thA, in1=thB, op=OP.add)
                V.scalar_tensor_tensor(out=uu[:], in0=thA, scalar=2.0, in1=thB,
                                       op0=OP.mult, op1=OP.add)   # 2ta+tb
                V.tensor_tensor(out=vv[:], in0=thB, in1=thA, op=OP.subtract)  # tb-ta

                # f0 = EA/L * (ua - ub); f3 = -f0
                V.tensor_tensor(out=du[:], in0=dlocA[:, 0, :], in1=dlocB[:, 0, :], op=OP.subtract)
                V.tensor_tensor(out=floct[:, 0, :], in0=EAL[:], in1=du[:], op=OP.mult)
                S.mul(out=floct[:, 3, :], in_=floct[:, 0, :], mul=-1.0)

                # h = 6EI/L2*(wa-wb);  f1 = 2/L*h + 6EI/L2*(ta+tb) = 12EI/L3*(wa-wb)+...
                V.tensor_tensor(out=wab[:], in0=dlocA[:, 1, :], in1=dlocB[:, 1, :], op=OP.subtract)
                V.tensor_tensor(out=hh[:], in0=e2[:], in1=wab[:], op=OP.mult)
                V.tensor_tensor(out=t1[:], in0=inv[:], in1=hh[:], op=OP.mult)
                V.tensor_tensor(out=t2[:], in0=e2[:], in1=tsum[:], op=OP.mult)
                V.scalar_tensor_tensor(out=floct[:, 1, :], in0=t1[:], scalar=2.0, in1=t2[:],
                                       op0=OP.mult, op1=OP.add)
                S.mul(out=floct[:, 4, :], in_=floct[:, 1, :], mul=-1.0)

                # M_mid = EI/L*(tb-ta) = (f5-f2)/2
                V.tensor_tensor(out=mmt[:], in0=EIL[:], in1=vv[:], op=OP.mult)
                # f2 = h + (2EI/L)*(2ta+tb);  f5 = f2 + 2*M_mid
                V.tensor_tensor(out=t1[:], in0=EX2[:], in1=uu[:], op=OP.mult)
                V.tensor_tensor(out=floct[:, 2, :], in0=hh[:], in1=t1[:], op=OP.add)
                V.scalar_tensor_tensor(out=floct[:, 5, :], in0=mmt[:], scalar=2.0,
                                       in1=floct[:, 2, :], op0=OP.mult, op1=OP.add)

                # F_global_A = [c*f0 - s*f1, s*f0 + c*f1, f2]
                V.tensor_tensor(out=t1[:], in0=ct[:], in1=floct[:, 0, :], op=OP.mult)
                V.tensor_tensor(out=t2[:], in0=st[:], in1=floct[:, 1, :], op=OP.mult)
                V.tensor_tensor(out=FAt[:, 0, :], in0=t1[:], in1=t2[:], op=OP.subtract)
                V.tensor_tensor(out=t1[:], in0=st[:], in1=floct[:, 0, :], op=OP.mult)
                V.tensor_tensor(out=t2[:], in0=ct[:], in1=floct[:, 1, :], op=OP.mult)
                V.tensor_tensor(out=FAt[:, 1, :], in0=t1[:], in1=t2[:], op=OP.add)
                S.copy(out=FAt[:, 2, :], in_=floct[:, 2, :])
                # F_global_B = [-FA0, -FA1, f5] (exact: f3=-f0, f4=-f1)
                S.mul(out=FBt[:, 0, :], in_=FAt[:, 0, :], mul=-1.0)
                S.mul(out=FBt[:, 1, :], in_=FAt[:, 1, :], mul=-1.0)
                S.copy(out=FBt[:, 2, :], in_=floct[:, 5, :])

                # ---- store ----
                nc.scalar.dma_start(out=floc[:, :, sl], in_=floct[:])
                nc.scalar.dma_start(out=dloc[:, 0:3, sl], in_=dlocA[:])
                nc.scalar.dma_start(out=dloc[:, 3:6, sl], in_=dlocB[:])
                nc.scalar.dma_start(out=FAo[:, :, sl], in_=FAt[:])
                nc.scalar.dma_start(out=FBo[:, :, sl], in_=FBt[:])
                nc.scalar.dma_start(out=l0o[:, sl], in_=l0t[:])
                nc.scalar.dma_start(out=co[:, sl], in_=ct[:])
                nc.scalar.dma_start(out=so[:, sl], in_=st[:])
                nc.scalar.dma_start(out=mmo[:, sl], in_=mmt[:])

    nc.compile()
    return nc


def _get_nc():
    global _CACHED_NC
    if _CACHED_NC is None:
        _CACHED_NC = _build_module()
    return _CACHED_NC


def kernel(pred_disp, connectivity, coords, prop_E, prop_A, prop_I22):
    global LAST_EXEC_NS, LAST_MEAN_EXEC_NS
    pred_disp = np.asarray(pred_disp, dtype=np.float32)
    coords = np.asarray(coords, dtype=np.float32)
    conn = np.asarray(connectivity)
    prop_E = np.asarray(prop_E, dtype=np.float32)
    prop_A = np.asarray(prop_A, dtype=np.float32)
    prop_I22 = np.asarray(prop_I22, dtype=np.float32)

    nA = conn[:, 0]
    nB = conn[:, 1]

    # ---- shard + marshal inputs (host): packed node rows expanded per edge ----
    node_pack = np.empty((N_NODES, 5), np.float32)
    node_pack[:, 0] = coords[:, 0]
    node_pack[:, 1] = coords[:, 2]
    node_pack[:, 2:5] = pred_disp

    gApad = np.zeros((N_CORES, E_PAD, 5), np.float32)
    gBpad = np.zeros((N_CORES, E_PAD, 5), np.float32)
    pEpad = np.ones((N_CORES, E_PAD), np.float32)
    pApad = np.ones((N_CORES, E_PAD), np.float32)
    pIpad = np.ones((N_CORES, E_PAD), np.float32)
    for c in range(N_CORES):
        cs, ce = c * E_CORE, (c + 1) * E_CORE
        np.take(node_pack, nA[cs:ce], axis=0, out=gApad[c, :E_CORE], mode="clip")
        np.take(node_pack, nB[cs:ce], axis=0, out=gBpad[c, :E_CORE], mode="clip")
        pEpad[c, :E_CORE] = prop_E[cs:ce]
        pApad[c, :E_CORE] = prop_A[cs:ce]
        pIpad[c, :E_CORE] = prop_I22[cs:ce]
    gBpad[:, E_CORE:, 0] = 1.0  # padding rows: dx=1, dz=0 -> finite everywhere

    in_maps = [
        {
            "gA": gApad[c].reshape(P, KTOT, 5),
            "gB": gBpad[c].reshape(P, KTOT, 5),
            "pE": pEpad[c].reshape(P, KTOT),
            "pA": pApad[c].reshape(P, KTOT),
            "pI": pIpad[c].reshape(P, KTOT),
        }
        for c in range(N_CORES)
    ]

    nc = _get_nc()
    res = run_bass_kernel_spmd(nc, in_maps, core_ids=list(range(N_CORES)),
                               trace=_PROFILE)
    LAST_EXEC_NS = res.exec_time_ns
    LAST_MEAN_EXEC_NS = res.mean_exec_time_ns

    # ---- unshard outputs (plane-major device layout -> row-major arrays) ----
    f_local = np.empty((N_ELEMS, 6), np.float32)
    d_local = np.empty((N_ELEMS, 6), np.float32)
    F_A = np.empty((N_ELEMS, 3), np.float32)
    F_B = np.empty((N_ELEMS, 3), np.float32)
    l0 = np.empty(N_ELEMS, np.float32)
    c_ = np.empty(N_ELEMS, np.float32)
    s_ = np.empty(N_ELEMS, np.float32)
    M_mid = np.empty(N_ELEMS, np.float32)
    for c in range(N_CORES):
        cs, ce = c * E_CORE, (c + 1) * E_CORE
        r = res.results[c]
        f_local[cs:ce] = r["floc"].transpose(0, 2, 1).reshape(E_PAD, 6)[:E_CORE]
        d_local[cs:ce] = r["dloc"].transpose(0, 2, 1).reshape(E_PAD, 6)[:E_CORE]
        F_A[cs:ce] = r["FAo"].transpose(0, 2, 1).reshape(E_PAD, 3)[:E_CORE]
        F_B[cs:ce] = r["FBo"].transpose(0, 2, 1).reshape(E_PAD, 3)[:E_CORE]
        l0[cs:ce] = r["l0o"].reshape(E_PAD)[:E_CORE]
        c_[cs:ce] = r["co"].reshape(E_PAD)[:E_CORE]
        s_[cs:ce] = r["so"].reshape(E_PAD)[:E_CORE]
        M_mid[cs:ce] = r["mmo"].reshape(E_PAD)[:E_CORE]

    # duplicate-column outputs (defined as copies of f_local columns)
    N_e = np.ascontiguousarray(f_local[:, 3])
    M1_e = np.ascontiguousarray(f_local[:, 2])
    M2_e = np.ascontiguousarray(f_local[:, 5])
    V_e = np.ascontiguousarray(f_local[:, 4])

    # all-reduce of the scattered nodal forces across edge shards
    nodal_forces = np.zeros((N_NODES, 3), np.float32)
    for comp in range(3):
        acc = np.bincount(nA, weights=F_A[:, comp], minlength=N_NODES)
        acc += np.bincount(nB, weights=F_B[:, comp], minlength=N_NODES)
        nodal_forces[:, comp] = acc

    return (nodal_forces, f_local, d_local, F_A, F_B,
            N_e, M_mid, V_e, M1_e, M2_e, l0, c_, s_)
